# revision 13
# baseline (speedup 1.0000x reference)
"""Continuous Game-of-Life Trainium2 kernel (v13: FWL-padded bands).

Reference computation (per batch image, cyclic 3x3 stencil):
    around = 8-neighbor sum of x (torus wrap)
    survive = sigmoid(10(around-1.5)) * sigmoid(10(3.5-around))
    birth   = sigmoid(10(around-2.5)) * sigmoid(10(3.5-around))
    out     = x*survive + (1-x)*birth

Algebraic simplification (BETA=10 transitions are >= 1.0 apart):
    s_c := sigmoid(10*around - 10*c)
    out ~= x*(s1.5 - s2.5) + (s2.5 - s3.5)    (max abs err 4.5e-5)

Optimization history (each step trace-verified on HW):
  v7  456us: SWDGE fp32 input + per-strip 1-row halo DMAs; stalled in
      17-51us chunks with the input stream latency-bound.
  v8c 343us: one contiguous 127-row SWDGE DMA per strip; top halo filled
      by an 8KB SBUF->SBUF copy from the previous strip's tile on the
      scalar HWDGE ring (its wait is pre-satisfied when ACT reaches it,
      and descriptor-gen overlaps sigmoid execution).  Output on the
      sync ring.  NOTE: big DRAM->SBUF transfers MUST be SWDGE
      (nc.gpsimd) -- the HWDGE path lands the whole transfer on a
      single SDMA engine (~27 GB/s; measured 1.35ms kernel).
  v9  281us: host pre-casts x to fp16 -- halves input DMA bytes and
      deletes the on-chip DVE cast (on-chip math is bit-identical).
  v11 277us: software-pipelined tail (sub/mul/add/out of strip t-1
      emitted after strip t's sigmoids) + deeper pools.  A PWL-on-DVE
      s15 variant was tried and REVERTED: any DVE op reading PSUM
      closes a PE<->DVE cycle through PSUM bank recycling (540us).
      Offloading the final add to GPSIMD also regressed (Q7 tensor ops
      run at ~0.42 efficiency; 304us).
  v13: stationary band matrices zero-padded from 126 to 128 columns.
      FWL (fast weight load) requires NumWeights==128; with 126-column
      stationaries half the strips ran LDWEIGHTS-serialized matmuls
      (427ns vs 217ns per 512-col matmul), and the PE tail ate a
      ~1.9us/strip bubble in the ACT stream.

Per-strip engines (steady state ~6us/strip):
  - TensorE: 8-neighbor sum via banded fp16 matmuls accumulated in
    PSUM, grouped by stationary operand (m0 vertical-only on the center
    columns, then m1 3-tap on the left/right shifted column views).
  - ScalarE: three sigmoids straight out of PSUM (scale/bias fused),
    ~1.9us each -- the throughput wall of this kernel.
  - VectorE: double-width fp16 sub, mul, add.
  - DMA out: fp16 (host upcasts to fp32).

Sharding: pure data-parallel over batch: 16 images -> 8 cores x 2 images.
The torus wrap is per-image so there is no cross-core halo at all.
"""

import numpy as np

B, H, W = 16, 2048, 2048
N_CORES = 8
B_PER = B // N_CORES  # 2 images per core
STRIDE = 126  # output rows per strip (128 input rows incl. halos)
N_STRIPS = (H + STRIDE - 1) // STRIDE  # 17
NBANKS = W // 512  # PSUM 512-col chunks per strip
PCOLS = 128  # stationary free dim, zero-padded to 128 so FWL engages

_cached_nc = None


def _band_matrices(m, dtype=np.float16):
    """[m+2, 128] stationary operands for the vertical taps.

    Tile layout: partitions 0..m-1 hold image rows r0..r0+m-1 (the cells),
    partition m holds the bottom halo row r0+m, partition m+1 holds the top
    halo row r0-1.  For output row p the vertical neighbors are partitions
    p-1 (or m+1 when p==0) and p+1.

    m0[k, p] = 1 for the two vertical neighbors (no center),
    m1[k, p] = 1 for the full 3-tap (used on the column-shifted views).
    Columns m..127 are zero padding (garbage PSUM rows m..127): FWL
    (2-elements-per-read weight load) only engages at 128 columns.
    """
    m0 = np.zeros((m + 2, PCOLS), dtype)
    m1 = np.zeros((m + 2, PCOLS), dtype)
    for p in range(m):
        up = m + 1 if p == 0 else p - 1
        m0[up, p] = 1.0
        m0[p + 1, p] = 1.0
        m1[up, p] = 1.0
        m1[p, p] = 1.0
        m1[p + 1, p] = 1.0
    return m0, m1


def _build(b_per=B_PER, h=H, w=W, stride=STRIDE):
    global _cached_nc
    if _cached_nc is not None and (b_per, h, w, stride) == (B_PER, H, W, STRIDE):
        return _cached_nc

    import concourse.mybir as mybir
    from concourse.bacc import Bacc
    from concourse.tile import TileContext

    B_PER_, H_, W_, STRIDE_ = b_per, h, w, stride
    N_STRIPS_ = (H_ + STRIDE_ - 1) // STRIDE_
    NBANKS_ = W_ // 512
    KROWS = STRIDE_ + 2  # input rows per full strip

    f32 = mybir.dt.float32
    f16 = mybir.dt.float16
    Sig = mybir.ActivationFunctionType.Sigmoid

    nc = Bacc(trn_type="TRN2")
    x_d = nc.dram_tensor("x", [B_PER_, H_, W_], f16, kind="ExternalInput")
    y_d = nc.dram_tensor("y", [B_PER_, H_, W_], f16, kind="ExternalOutput")

    consts = {}
    for m in sorted({STRIDE_, H_ - STRIDE_ * (N_STRIPS_ - 1)}):
        m0_np, m1_np = _band_matrices(m)
        consts[m] = (
            nc.inline_tensor(m0_np, f"m0_const_{m}"),
            nc.inline_tensor(m1_np, f"m1_const_{m}"),
        )

    with TileContext(nc) as tc:
        with (
            tc.tile_pool(name="wpool", bufs=1) as wpool,
            tc.tile_pool(name="fpool", bufs=6) as fpool,
            tc.tile_pool(name="spool", bufs=4) as spool,
            tc.tile_pool(name="dpool", bufs=4) as dpool,
            tc.tile_pool(name="mpool", bufs=3) as mpool,
            tc.tile_pool(name="opool", bufs=6) as opool,
            tc.tile_pool(name="ppool", bufs=2, space="PSUM") as ppool,
        ):
            bands = {}
            for m, (m0_d, m1_d) in consts.items():
                m0 = wpool.tile([m + 2, PCOLS], f16, name=f"m0_{m}")
                m1 = wpool.tile([m + 2, PCOLS], f16, name=f"m1_{m}")
                nc.sync.dma_start(out=m0[:], in_=m0_d[:])
                nc.sync.dma_start(out=m1[:], in_=m1_d[:])
                bands[m] = (m0, m1)

            # activation biases must be [128,1] APs, not immediates
            b15 = wpool.tile([128, 1], f32)
            b25 = wpool.tile([128, 1], f32)
            b35 = wpool.tile([128, 1], f32)
            nc.vector.memset(b15[:], -15.0)
            nc.vector.memset(b25[:], -25.0)
            nc.vector.memset(b35[:], -35.0)

            prev_xf = None
            pending = None  # (sall, xf, M, b, r0) awaiting sub/mul/add/out

            def emit_tail(p):
                sall, xf, M, b, r0 = p
                de = dpool.tile([STRIDE_, 2 * W_], f16, tag="de")
                nc.vector.tensor_sub(
                    out=de[:M], in0=sall[:M, 0 : 2 * W_], in1=sall[:M, W_ : 3 * W_]
                )
                m_t = mpool.tile([STRIDE_, W_], f16, tag="m")
                o = opool.tile([STRIDE_, W_], f16, tag="o")
                nc.vector.tensor_mul(out=m_t[:M], in0=xf[:M, :], in1=de[:M, 0:W_])
                nc.vector.tensor_add(out=o[:M], in0=m_t[:M], in1=de[:M, W_ : 2 * W_])
                nc.sync.dma_start(out=y_d[b, r0 : r0 + M, :], in_=o[:M])

            for b in range(B_PER_):
                for t in range(N_STRIPS_):
                    r0 = t * STRIDE_  # first output row
                    M = min(STRIDE_, H_ - r0)  # output rows this strip
                    k = M + 2  # partitions used (cells + 2 halos)
                    m0, m1 = bands[M]

                    # fp16 tile (host pre-casts x), partitions 0..M-1 =
                    # cells (rows r0..), partition M = bottom halo, M+1 =
                    # top halo.
                    xf = fpool.tile([KROWS, W_], f16, tag="xf")
                    if r0 + M < H_:
                        # cells + bottom halo: one contiguous SWDGE DMA
                        # (HWDGE DRAM->SBUF lands on a single SDMA engine
                        # -- measured 27 GB/s; SWDGE sprays all 16).
                        nc.gpsimd.dma_start(
                            out=xf[0 : M + 1, :], in_=x_d[b, r0 : r0 + M + 1, :]
                        )
                    else:
                        # last strip: bottom halo wraps to row 0
                        nc.gpsimd.dma_start(out=xf[0:M, :], in_=x_d[b, r0:H_, :])
                        nc.gpsimd.dma_start(out=xf[M : M + 1, :], in_=x_d[b, 0:1, :])
                    if t == 0:
                        # top halo wraps to the last image row (rare: 2/image)
                        nc.gpsimd.dma_start(
                            out=xf[M + 1 : M + 2, :], in_=x_d[b, H_ - 1 : H_, :]
                        )
                    else:
                        # top halo row r0-1 = previous strip's partition
                        # STRIDE-1: an 8KB SBUF->SBUF copy on the scalar
                        # HWDGE ring.  Its wait (previous strip's input
                        # landed) is long satisfied when the ACT queue
                        # reaches it, and its descriptor-gen overlaps
                        # sigmoid execution, so it costs nothing.
                        nc.scalar.dma_start(
                            out=xf[M + 1 : M + 2, :],
                            in_=prev_xf[STRIDE_ - 1 : STRIDE_, :],
                        )
                    prev_xf = xf

                    ps = ppool.tile([PCOLS, W_], f32, tag="ps")
                    m0s = m0[:k, :PCOLS]
                    m1s = m1[:k, :PCOLS]

                    # Pre-touch: a 1x1 matmul absorbs the PSUM-release wait
                    # (Matmult carries at most ONE sync wait; without this,
                    # Bacc's wait-merging couples strip t to strip t-1's
                    # activations and serializes PE behind ACT).
                    nc.tensor.matmul(
                        ps[:1, 0:1], b15[:1, :1], b15[:1, :1],
                        start=True, stop=True,
                    )

                    # around = 8-neighbor sum accumulated in PSUM, grouped
                    # by stationary operand to minimize weight switches.
                    # m0 group: center column, vertical neighbors only.
                    for nb in range(NBANKS_):
                        c0 = nb * 512
                        nc.tensor.matmul(
                            ps[:PCOLS, c0 : c0 + 512], m0s, xf[:k, c0 : c0 + 512],
                            start=True, stop=False,
                        )
                    # m1 group, left-neighbor column: out col j += band @ x col j-1
                    for nb in range(NBANKS_):
                        c0 = nb * 512
                        c1 = c0 + 512
                        if nb == 0:
                            nc.tensor.matmul(
                                ps[:PCOLS, 1:512], m1s, xf[:k, 0:511],
                                start=False, stop=False,
                            )
                            nc.tensor.matmul(
                                ps[:PCOLS, 0:1], m1s, xf[:k, W_ - 1 : W_],
                                start=False, stop=False,
                            )
                        else:
                            nc.tensor.matmul(
                                ps[:PCOLS, c0:c1], m1s, xf[:k, c0 - 1 : c1 - 1],
                                start=False, stop=False,
                            )
                    # m1 group, right-neighbor column: out col j += band @ x col j+1
                    for nb in range(NBANKS_):
                        c0 = nb * 512
                        c1 = c0 + 512
                        if nb == NBANKS_ - 1:
                            nc.tensor.matmul(
                                ps[:PCOLS, c0 : W_ - 1], m1s, xf[:k, c0 + 1 : W_],
                                start=False, stop=False,
                            )
                            nc.tensor.matmul(
                                ps[:PCOLS, W_ - 1 : W_], m1s, xf[:k, 0:1],
                                start=False, stop=True,
                            )
                        else:
                            nc.tensor.matmul(
                                ps[:PCOLS, c0:c1], m1s, xf[:k, c0 + 1 : c1 + 1],
                                start=False, stop=True,
                            )

                    # one contiguous tile [s15 | s25 | s35] so a single
                    # double-width DVE sub computes d=s15-s25 and e=s25-s35
                    # via overlapping slices
                    sall = spool.tile([STRIDE_, 3 * W_], f16, tag="sall")
                    nc.scalar.activation(sall[:M, 0:W_], ps[:M], Sig, bias=b15[:M], scale=10.0)
                    nc.scalar.activation(sall[:M, W_ : 2 * W_], ps[:M], Sig, bias=b25[:M], scale=10.0)
                    nc.scalar.activation(sall[:M, 2 * W_ : 3 * W_], ps[:M], Sig, bias=b35[:M], scale=10.0)

                    # Software pipelining: the sub/mul/add/out for the
                    # PREVIOUS strip are emitted here so no DVE-stream wait
                    # chains across pipeline stages.
                    if pending is not None:
                        emit_tail(pending)
                    pending = (sall, xf, M, b, r0)

            if pending is not None:
                emit_tail(pending)

    nc.compile()
    if (b_per, h, w, stride) == (B_PER, H, W, STRIDE):
        _cached_nc = nc
    return nc


def run(x, trace=False):
    """Run the SPMD kernel on 8 cores. Returns (out_fp32, BassKernelResults)."""
    from concourse.bass_utils import run_bass_kernel_spmd

    nc = _build()
    x = np.asarray(x, dtype=np.float16)  # host-side cast: halves input DMA
    assert x.shape == (B, H, W), x.shape
    in_maps = [{"x": x[B_PER * c : B_PER * (c + 1)]} for c in range(N_CORES)]
    res = run_bass_kernel_spmd(nc, in_maps, core_ids=list(range(N_CORES)), trace=trace)
    out = np.concatenate(
        [res.results[c]["y"].astype(np.float32) for c in range(N_CORES)], axis=0
    )
    return out, res


def kernel(x):
    out, _ = run(x, trace=False)
    return out


# revision 17
# speedup vs baseline: 1.5644x; 1.5644x over previous
"""Continuous Game-of-Life Trainium2 kernel (v13: FWL-padded bands).

Reference computation (per batch image, cyclic 3x3 stencil):
    around = 8-neighbor sum of x (torus wrap)
    survive = sigmoid(10(around-1.5)) * sigmoid(10(3.5-around))
    birth   = sigmoid(10(around-2.5)) * sigmoid(10(3.5-around))
    out     = x*survive + (1-x)*birth

Algebraic simplification (BETA=10 transitions are >= 1.0 apart):
    s_c := sigmoid(10*around - 10*c)
    out ~= x*(s1.5 - s2.5) + (s2.5 - s3.5)    (max abs err 4.5e-5)

Optimization history (each step trace-verified on HW):
  v7  456us: SWDGE fp32 input + per-strip 1-row halo DMAs; stalled in
      17-51us chunks with the input stream latency-bound.
  v8c 343us: one contiguous 127-row SWDGE DMA per strip; top halo filled
      by an 8KB SBUF->SBUF copy from the previous strip's tile on the
      scalar HWDGE ring (its wait is pre-satisfied when ACT reaches it,
      and descriptor-gen overlaps sigmoid execution).  Output on the
      sync ring.  NOTE: big DRAM->SBUF transfers MUST be SWDGE
      (nc.gpsimd) -- the HWDGE path lands the whole transfer on a
      single SDMA engine (~27 GB/s; measured 1.35ms kernel).
  v9  281us: host pre-casts x to fp16 -- halves input DMA bytes and
      deletes the on-chip DVE cast (on-chip math is bit-identical).
  v11 277us: software-pipelined tail (sub/mul/add/out of strip t-1
      emitted after strip t's sigmoids) + deeper pools.  A PWL-on-DVE
      s15 variant was tried and REVERTED: any DVE op reading PSUM
      closes a PE<->DVE cycle through PSUM bank recycling (540us).
      Offloading the final add to GPSIMD also regressed (Q7 tensor ops
      run at ~0.42 efficiency; 304us).
  v13: stationary band matrices zero-padded from 126 to 128 columns.
      FWL (fast weight load) requires NumWeights==128; with 126-column
      stationaries half the strips ran LDWEIGHTS-serialized matmuls
      (427ns vs 217ns per 512-col matmul), and the PE tail ate a
      ~1.9us/strip bubble in the ACT stream.

Per-strip engines (steady state ~6us/strip):
  - TensorE: 8-neighbor sum via banded fp16 matmuls accumulated in
    PSUM, grouped by stationary operand (m0 vertical-only on the center
    columns, then m1 3-tap on the left/right shifted column views).
  - ScalarE: three sigmoids straight out of PSUM (scale/bias fused),
    ~1.9us each -- the throughput wall of this kernel.
  - VectorE: double-width fp16 sub, mul, add.
  - DMA out: fp16 (host upcasts to fp32).

Sharding: pure data-parallel over batch: 16 images -> 8 cores x 2 images.
The torus wrap is per-image so there is no cross-core halo at all.
"""

import numpy as np

B, H, W = 16, 2048, 2048
N_CORES = 8
B_PER = B // N_CORES  # 2 images per core
STRIDE = 126  # output rows per strip (128 input rows incl. halos)
N_STRIPS = (H + STRIDE - 1) // STRIDE  # 17
NBANKS = W // 512  # PSUM 512-col chunks per strip
PCOLS = 128  # stationary free dim, zero-padded to 128 so FWL engages

_cached_nc = None


def _band_matrices(m, dtype=np.float16):
    """[m+2, 128] stationary operands for the vertical taps.

    Tile layout: partitions 0..m-1 hold image rows r0..r0+m-1 (the cells),
    partition m holds the bottom halo row r0+m, partition m+1 holds the top
    halo row r0-1.  For output row p the vertical neighbors are partitions
    p-1 (or m+1 when p==0) and p+1.

    m0[k, p] = 1 for the two vertical neighbors (no center),
    m1[k, p] = 1 for the full 3-tap (used on the column-shifted views).
    Columns m..127 are zero padding (garbage PSUM rows m..127): FWL
    (2-elements-per-read weight load) only engages at 128 columns.
    """
    m0 = np.zeros((m + 2, PCOLS), dtype)
    m1 = np.zeros((m + 2, PCOLS), dtype)
    for p in range(m):
        up = m + 1 if p == 0 else p - 1
        m0[up, p] = 1.0
        m0[p + 1, p] = 1.0
        m1[up, p] = 1.0
        m1[p, p] = 1.0
        m1[p + 1, p] = 1.0
    return m0, m1


def _build(b_per=B_PER, h=H, w=W, stride=STRIDE):
    global _cached_nc
    if _cached_nc is not None and (b_per, h, w, stride) == (B_PER, H, W, STRIDE):
        return _cached_nc

    import concourse.mybir as mybir
    from concourse.bacc import Bacc
    from concourse.tile import TileContext

    B_PER_, H_, W_, STRIDE_ = b_per, h, w, stride
    N_STRIPS_ = (H_ + STRIDE_ - 1) // STRIDE_
    NBANKS_ = W_ // 512
    KROWS = STRIDE_ + 2  # input rows per full strip

    f32 = mybir.dt.float32
    f16 = mybir.dt.float16
    Sig = mybir.ActivationFunctionType.Sigmoid

    nc = Bacc(trn_type="TRN2")
    x_d = nc.dram_tensor("x", [B_PER_, H_, W_], f16, kind="ExternalInput")
    y_d = nc.dram_tensor("y", [B_PER_, H_, W_], f16, kind="ExternalOutput")

    consts = {}
    for m in sorted({STRIDE_, H_ - STRIDE_ * (N_STRIPS_ - 1)}):
        m0_np, m1_np = _band_matrices(m)
        consts[m] = (
            nc.inline_tensor(m0_np, f"m0_const_{m}"),
            nc.inline_tensor(m1_np, f"m1_const_{m}"),
        )

    with TileContext(nc) as tc:
        with (
            tc.tile_pool(name="wpool", bufs=1) as wpool,
            tc.tile_pool(name="fpool", bufs=12) as fpool,
            tc.tile_pool(name="spool", bufs=4) as spool,
            tc.tile_pool(name="dpool", bufs=4) as dpool,
            tc.tile_pool(name="mpool", bufs=3) as mpool,
            tc.tile_pool(name="opool", bufs=6) as opool,
            tc.tile_pool(name="ppool", bufs=2, space="PSUM") as ppool,
        ):
            bands = {}
            for m, (m0_d, m1_d) in consts.items():
                m0 = wpool.tile([m + 2, PCOLS], f16, name=f"m0_{m}")
                m1 = wpool.tile([m + 2, PCOLS], f16, name=f"m1_{m}")
                nc.sync.dma_start(out=m0[:], in_=m0_d[:])
                nc.sync.dma_start(out=m1[:], in_=m1_d[:])
                bands[m] = (m0, m1)

            # activation biases must be [128,1] APs, not immediates
            b15 = wpool.tile([128, 1], f32)
            b25 = wpool.tile([128, 1], f32)
            b35 = wpool.tile([128, 1], f32)
            nc.vector.memset(b15[:], -15.0)
            nc.vector.memset(b25[:], -25.0)
            nc.vector.memset(b35[:], -35.0)

            prev_xf = None
            pending = None  # (sall, xf, M, b, r0) awaiting sub/mul/add/out

            def emit_tail(p):
                sall, xf, M, b, r0 = p
                de = dpool.tile([STRIDE_, 2 * W_], f16, tag="de")
                nc.vector.tensor_sub(
                    out=de[:M], in0=sall[:M, 0 : 2 * W_], in1=sall[:M, W_ : 3 * W_]
                )
                m_t = mpool.tile([STRIDE_, W_], f16, tag="m")
                o = opool.tile([STRIDE_, W_], f16, tag="o")
                nc.vector.tensor_mul(out=m_t[:M], in0=xf[:M, :], in1=de[:M, 0:W_])
                nc.vector.tensor_add(out=o[:M], in0=m_t[:M], in1=de[:M, W_ : 2 * W_])
                nc.sync.dma_start(out=y_d[b, r0 : r0 + M, :], in_=o[:M])

            for b in range(B_PER_):
                for t in range(N_STRIPS_):
                    r0 = t * STRIDE_  # first output row
                    M = min(STRIDE_, H_ - r0)  # output rows this strip
                    k = M + 2  # partitions used (cells + 2 halos)
                    m0, m1 = bands[M]

                    # fp16 tile (host pre-casts x), partitions 0..M-1 =
                    # cells (rows r0..), partition M = bottom halo, M+1 =
                    # top halo.
                    xf = fpool.tile([KROWS, W_], f16, tag="xf")
                    if r0 + M < H_:
                        # cells + bottom halo: one contiguous SWDGE DMA
                        # (HWDGE DRAM->SBUF lands on a single SDMA engine
                        # -- measured 27 GB/s; SWDGE sprays all 16).
                        nc.gpsimd.dma_start(
                            out=xf[0 : M + 1, :], in_=x_d[b, r0 : r0 + M + 1, :]
                        )
                    else:
                        # last strip: bottom halo wraps to row 0
                        nc.gpsimd.dma_start(out=xf[0:M, :], in_=x_d[b, r0:H_, :])
                        nc.gpsimd.dma_start(out=xf[M : M + 1, :], in_=x_d[b, 0:1, :])
                    if t == 0:
                        # top halo wraps to the last image row (rare: 2/image)
                        nc.gpsimd.dma_start(
                            out=xf[M + 1 : M + 2, :], in_=x_d[b, H_ - 1 : H_, :]
                        )
                    else:
                        # top halo row r0-1 = previous strip's partition
                        # STRIDE-1: an 8KB SBUF->SBUF copy on the scalar
                        # HWDGE ring.  Its wait (previous strip's input
                        # landed) is long satisfied when the ACT queue
                        # reaches it, and its descriptor-gen overlaps
                        # sigmoid execution, so it costs nothing.
                        nc.scalar.dma_start(
                            out=xf[M + 1 : M + 2, :],
                            in_=prev_xf[STRIDE_ - 1 : STRIDE_, :],
                        )
                    prev_xf = xf

                    ps = ppool.tile([PCOLS, W_], f32, tag="ps")
                    m0s = m0[:k, :PCOLS]
                    m1s = m1[:k, :PCOLS]

                    # Pre-touch: a 1x1 matmul absorbs the PSUM-release wait
                    # (Matmult carries at most ONE sync wait; without this,
                    # Bacc's wait-merging couples strip t to strip t-1's
                    # activations and serializes PE behind ACT).
                    nc.tensor.matmul(
                        ps[:1, 0:1], b15[:1, :1], b15[:1, :1],
                        start=True, stop=True,
                    )

                    # around = 8-neighbor sum accumulated in PSUM, grouped
                    # by stationary operand to minimize weight switches.
                    # (N=512 is a hard matmul limit: wider PSUM APs fail
                    # walrus codegen's s3d3_mm_num_elements check.)
                    # m0 group: center column, vertical neighbors only.
                    for nb in range(NBANKS_):
                        c0 = nb * 512
                        nc.tensor.matmul(
                            ps[:PCOLS, c0 : c0 + 512], m0s, xf[:k, c0 : c0 + 512],
                            start=True, stop=False,
                        )
                    # m1 group, left-neighbor column: out col j += band @ x col j-1
                    for nb in range(NBANKS_):
                        c0 = nb * 512
                        c1 = c0 + 512
                        if nb == 0:
                            nc.tensor.matmul(
                                ps[:PCOLS, 1:512], m1s, xf[:k, 0:511],
                                start=False, stop=False,
                            )
                            nc.tensor.matmul(
                                ps[:PCOLS, 0:1], m1s, xf[:k, W_ - 1 : W_],
                                start=False, stop=False,
                            )
                        else:
                            nc.tensor.matmul(
                                ps[:PCOLS, c0:c1], m1s, xf[:k, c0 - 1 : c1 - 1],
                                start=False, stop=False,
                            )
                    # m1 group, right-neighbor column: out col j += band @ x col j+1
                    for nb in range(NBANKS_):
                        c0 = nb * 512
                        c1 = c0 + 512
                        if nb == NBANKS_ - 1:
                            nc.tensor.matmul(
                                ps[:PCOLS, c0 : W_ - 1], m1s, xf[:k, c0 + 1 : W_],
                                start=False, stop=False,
                            )
                            nc.tensor.matmul(
                                ps[:PCOLS, W_ - 1 : W_], m1s, xf[:k, 0:1],
                                start=False, stop=True,
                            )
                        else:
                            nc.tensor.matmul(
                                ps[:PCOLS, c0:c1], m1s, xf[:k, c0 + 1 : c1 + 1],
                                start=False, stop=True,
                            )

                    # one contiguous tile [s15 | s25 | s35] so a single
                    # double-width DVE sub computes d=s15-s25 and e=s25-s35
                    # via overlapping slices
                    sall = spool.tile([STRIDE_, 3 * W_], f16, tag="sall")
                    nc.scalar.activation(sall[:M, 0:W_], ps[:M], Sig, bias=b15[:M], scale=10.0)
                    nc.scalar.activation(sall[:M, W_ : 2 * W_], ps[:M], Sig, bias=b25[:M], scale=10.0)
                    nc.scalar.activation(sall[:M, 2 * W_ : 3 * W_], ps[:M], Sig, bias=b35[:M], scale=10.0)

                    # Software pipelining: the sub/mul/add/out for the
                    # PREVIOUS strip are emitted here so no DVE-stream wait
                    # chains across pipeline stages.
                    if pending is not None:
                        emit_tail(pending)
                    pending = (sall, xf, M, b, r0)

            if pending is not None:
                emit_tail(pending)

    nc.compile()
    if (b_per, h, w, stride) == (B_PER, H, W, STRIDE):
        _cached_nc = nc
    return nc


def run(x, trace=False):
    """Run the SPMD kernel on 8 cores. Returns (out_fp32, BassKernelResults)."""
    from concourse.bass_utils import run_bass_kernel_spmd

    nc = _build()
    x = np.asarray(x, dtype=np.float16)  # host-side cast: halves input DMA
    assert x.shape == (B, H, W), x.shape
    in_maps = [{"x": x[B_PER * c : B_PER * (c + 1)]} for c in range(N_CORES)]
    res = run_bass_kernel_spmd(nc, in_maps, core_ids=list(range(N_CORES)), trace=trace)
    out = np.concatenate(
        [res.results[c]["y"].astype(np.float32) for c in range(N_CORES)], axis=0
    )
    return out, res


def kernel(x):
    out, _ = run(x, trace=False)
    return out


# revision 18
# speedup vs baseline: 1.5895x; 1.0161x over previous
"""Continuous Game-of-Life Trainium2 kernel (v13: FWL-padded bands).

Reference computation (per batch image, cyclic 3x3 stencil):
    around = 8-neighbor sum of x (torus wrap)
    survive = sigmoid(10(around-1.5)) * sigmoid(10(3.5-around))
    birth   = sigmoid(10(around-2.5)) * sigmoid(10(3.5-around))
    out     = x*survive + (1-x)*birth

Algebraic simplification (BETA=10 transitions are >= 1.0 apart):
    s_c := sigmoid(10*around - 10*c)
    out ~= x*(s1.5 - s2.5) + (s2.5 - s3.5)    (max abs err 4.5e-5)

Optimization history (each step trace-verified on HW):
  v7  456us: SWDGE fp32 input + per-strip 1-row halo DMAs; stalled in
      17-51us chunks with the input stream latency-bound.
  v8c 343us: one contiguous 127-row SWDGE DMA per strip; top halo filled
      by an 8KB SBUF->SBUF copy from the previous strip's tile on the
      scalar HWDGE ring (its wait is pre-satisfied when ACT reaches it,
      and descriptor-gen overlaps sigmoid execution).  Output on the
      sync ring.  NOTE: big DRAM->SBUF transfers MUST be SWDGE
      (nc.gpsimd) -- the HWDGE path lands the whole transfer on a
      single SDMA engine (~27 GB/s; measured 1.35ms kernel).
  v9  281us: host pre-casts x to fp16 -- halves input DMA bytes and
      deletes the on-chip DVE cast (on-chip math is bit-identical).
  v11 277us: software-pipelined tail (sub/mul/add/out of strip t-1
      emitted after strip t's sigmoids) + deeper pools.  A PWL-on-DVE
      s15 variant was tried and REVERTED: any DVE op reading PSUM
      closes a PE<->DVE cycle through PSUM bank recycling (540us).
      Offloading the final add to GPSIMD also regressed (Q7 tensor ops
      run at ~0.42 efficiency; 304us).
  v13: stationary band matrices zero-padded from 126 to 128 columns.
      FWL (fast weight load) requires NumWeights==128; with 126-column
      stationaries half the strips ran LDWEIGHTS-serialized matmuls
      (427ns vs 217ns per 512-col matmul), and the PE tail ate a
      ~1.9us/strip bubble in the ACT stream.

Per-strip engines (steady state ~6us/strip):
  - TensorE: 8-neighbor sum via banded fp16 matmuls accumulated in
    PSUM, grouped by stationary operand (m0 vertical-only on the center
    columns, then m1 3-tap on the left/right shifted column views).
  - ScalarE: three sigmoids straight out of PSUM (scale/bias fused),
    ~1.9us each -- the throughput wall of this kernel.
  - VectorE: double-width fp16 sub, mul, add.
  - DMA out: fp16 (host upcasts to fp32).

Sharding: pure data-parallel over batch: 16 images -> 8 cores x 2 images.
The torus wrap is per-image so there is no cross-core halo at all.
"""

import numpy as np

B, H, W = 16, 2048, 2048
N_CORES = 8
B_PER = B // N_CORES  # 2 images per core
STRIDE = 126  # output rows per strip (128 input rows incl. halos)
N_STRIPS = (H + STRIDE - 1) // STRIDE  # 17
NBANKS = W // 512  # PSUM 512-col chunks per strip
PCOLS = 128  # stationary free dim, zero-padded to 128 so FWL engages

_cached_nc = None


def _band_matrices(m, dtype=np.float16):
    """[m+2, 128] stationary operands for the vertical taps.

    Tile layout: partitions 0..m-1 hold image rows r0..r0+m-1 (the cells),
    partition m holds the bottom halo row r0+m, partition m+1 holds the top
    halo row r0-1.  For output row p the vertical neighbors are partitions
    p-1 (or m+1 when p==0) and p+1.

    m0[k, p] = 1 for the two vertical neighbors (no center),
    m1[k, p] = 1 for the full 3-tap (used on the column-shifted views).
    Columns m..127 are zero padding (garbage PSUM rows m..127): FWL
    (2-elements-per-read weight load) only engages at 128 columns.
    """
    m0 = np.zeros((m + 2, PCOLS), dtype)
    m1 = np.zeros((m + 2, PCOLS), dtype)
    for p in range(m):
        up = m + 1 if p == 0 else p - 1
        m0[up, p] = 1.0
        m0[p + 1, p] = 1.0
        m1[up, p] = 1.0
        m1[p, p] = 1.0
        m1[p + 1, p] = 1.0
    return m0, m1


def _build(b_per=B_PER, h=H, w=W, stride=STRIDE):
    global _cached_nc
    if _cached_nc is not None and (b_per, h, w, stride) == (B_PER, H, W, STRIDE):
        return _cached_nc

    import concourse.mybir as mybir
    from concourse.bacc import Bacc
    from concourse.tile import TileContext

    B_PER_, H_, W_, STRIDE_ = b_per, h, w, stride
    N_STRIPS_ = (H_ + STRIDE_ - 1) // STRIDE_
    NBANKS_ = W_ // 512
    KROWS = STRIDE_ + 2  # input rows per full strip

    f32 = mybir.dt.float32
    f16 = mybir.dt.float16
    Sig = mybir.ActivationFunctionType.Sigmoid

    nc = Bacc(trn_type="TRN2")
    x_d = nc.dram_tensor("x", [B_PER_, H_, W_], f16, kind="ExternalInput")
    y_d = nc.dram_tensor("y", [B_PER_, H_, W_], f16, kind="ExternalOutput")

    consts = {}
    for m in sorted({STRIDE_, H_ - STRIDE_ * (N_STRIPS_ - 1)}):
        m0_np, m1_np = _band_matrices(m)
        consts[m] = (
            nc.inline_tensor(m0_np, f"m0_const_{m}"),
            nc.inline_tensor(m1_np, f"m1_const_{m}"),
        )

    with TileContext(nc) as tc:
        with (
            tc.tile_pool(name="wpool", bufs=1) as wpool,
            tc.tile_pool(name="fpool", bufs=12) as fpool,
            tc.tile_pool(name="spool", bufs=4) as spool,
            tc.tile_pool(name="dpool", bufs=4) as dpool,
            tc.tile_pool(name="mpool", bufs=3) as mpool,
            tc.tile_pool(name="opool", bufs=6) as opool,
            tc.tile_pool(name="ppool", bufs=2, space="PSUM") as ppool,
        ):
            bands = {}
            for m, (m0_d, m1_d) in consts.items():
                m0 = wpool.tile([m + 2, PCOLS], f16, name=f"m0_{m}")
                m1 = wpool.tile([m + 2, PCOLS], f16, name=f"m1_{m}")
                nc.sync.dma_start(out=m0[:], in_=m0_d[:])
                nc.sync.dma_start(out=m1[:], in_=m1_d[:])
                bands[m] = (m0, m1)

            # activation biases must be [128,1] APs, not immediates
            b15 = wpool.tile([128, 1], f32)
            b25 = wpool.tile([128, 1], f32)
            b35 = wpool.tile([128, 1], f32)
            nc.vector.memset(b15[:], -15.0)
            nc.vector.memset(b25[:], -25.0)
            nc.vector.memset(b35[:], -35.0)

            prev_xf = None
            pending = None  # (sall, xf, M, b, r0) awaiting sub/mul/add/out

            def emit_tail(p):
                sall, xf, M, b, r0 = p
                de = dpool.tile([STRIDE_, 2 * W_], f16, tag="de")
                nc.vector.tensor_sub(
                    out=de[:M], in0=sall[:M, 0 : 2 * W_], in1=sall[:M, W_ : 3 * W_]
                )
                m_t = mpool.tile([STRIDE_, W_], f16, tag="m")
                o = opool.tile([STRIDE_, W_], f16, tag="o")
                nc.vector.tensor_mul(out=m_t[:M], in0=xf[:M, :], in1=de[:M, 0:W_])
                nc.vector.tensor_add(out=o[:M], in0=m_t[:M], in1=de[:M, W_ : 2 * W_])
                nc.sync.dma_start(out=y_d[b, r0 : r0 + M, :], in_=o[:M])

            for b in range(B_PER_):
                for t in range(N_STRIPS_):
                    r0 = t * STRIDE_  # first output row
                    M = min(STRIDE_, H_ - r0)  # output rows this strip
                    k = M + 2  # partitions used (cells + 2 halos)
                    m0, m1 = bands[M]

                    # fp16 tile (host pre-casts x), partitions 0..M-1 =
                    # cells (rows r0..), partition M = bottom halo, M+1 =
                    # top halo.
                    xf = fpool.tile([KROWS, W_], f16, tag="xf")
                    if r0 + M < H_:
                        # cells + bottom halo: one contiguous SWDGE DMA
                        # (HWDGE DRAM->SBUF lands on a single SDMA engine
                        # -- measured 27 GB/s; SWDGE sprays all 16).
                        nc.gpsimd.dma_start(
                            out=xf[0 : M + 1, :], in_=x_d[b, r0 : r0 + M + 1, :]
                        )
                    else:
                        # last strip: bottom halo wraps to row 0
                        nc.gpsimd.dma_start(out=xf[0:M, :], in_=x_d[b, r0:H_, :])
                        nc.gpsimd.dma_start(out=xf[M : M + 1, :], in_=x_d[b, 0:1, :])
                    if t == 0:
                        # top halo wraps to the last image row (rare: 2/image)
                        nc.gpsimd.dma_start(
                            out=xf[M + 1 : M + 2, :], in_=x_d[b, H_ - 1 : H_, :]
                        )
                    else:
                        # top halo row r0-1 = previous strip's partition
                        # STRIDE-1: an 8KB SBUF->SBUF copy on the scalar
                        # HWDGE ring.  Its wait (previous strip's input
                        # landed) is long satisfied when the ACT queue
                        # reaches it, and its descriptor-gen overlaps
                        # sigmoid execution, so it costs nothing.
                        nc.scalar.dma_start(
                            out=xf[M + 1 : M + 2, :],
                            in_=prev_xf[STRIDE_ - 1 : STRIDE_, :],
                        )
                    prev_xf = xf

                    ps = ppool.tile([PCOLS, W_], f32, tag="ps")
                    m0s = m0[:k, :PCOLS]
                    m1s = m1[:k, :PCOLS]

                    # Pre-touch: a 1x1 matmul absorbs the PSUM-release wait
                    # (Matmult carries at most ONE sync wait; without this,
                    # Bacc's wait-merging couples strip t to strip t-1's
                    # activations and serializes PE behind ACT).
                    nc.tensor.matmul(
                        ps[:1, 0:1], m0[:1, :1], m0[:1, :1],
                        start=True, stop=True,
                    )

                    # around = 8-neighbor sum accumulated in PSUM, grouped
                    # by stationary operand to minimize weight switches.
                    # (N=512 is a hard matmul limit: wider PSUM APs fail
                    # walrus codegen's s3d3_mm_num_elements check.)
                    # m0 group: center column, vertical neighbors only.
                    for nb in range(NBANKS_):
                        c0 = nb * 512
                        nc.tensor.matmul(
                            ps[:PCOLS, c0 : c0 + 512], m0s, xf[:k, c0 : c0 + 512],
                            start=True, stop=False,
                        )
                    # m1 group, left-neighbor column: out col j += band @ x col j-1
                    for nb in range(NBANKS_):
                        c0 = nb * 512
                        c1 = c0 + 512
                        if nb == 0:
                            nc.tensor.matmul(
                                ps[:PCOLS, 1:512], m1s, xf[:k, 0:511],
                                start=False, stop=False,
                            )
                            nc.tensor.matmul(
                                ps[:PCOLS, 0:1], m1s, xf[:k, W_ - 1 : W_],
                                start=False, stop=False,
                            )
                        else:
                            nc.tensor.matmul(
                                ps[:PCOLS, c0:c1], m1s, xf[:k, c0 - 1 : c1 - 1],
                                start=False, stop=False,
                            )
                    # m1 group, right-neighbor column: out col j += band @ x col j+1
                    for nb in range(NBANKS_):
                        c0 = nb * 512
                        c1 = c0 + 512
                        if nb == NBANKS_ - 1:
                            nc.tensor.matmul(
                                ps[:PCOLS, c0 : W_ - 1], m1s, xf[:k, c0 + 1 : W_],
                                start=False, stop=False,
                            )
                            nc.tensor.matmul(
                                ps[:PCOLS, W_ - 1 : W_], m1s, xf[:k, 0:1],
                                start=False, stop=True,
                            )
                        else:
                            nc.tensor.matmul(
                                ps[:PCOLS, c0:c1], m1s, xf[:k, c0 + 1 : c1 + 1],
                                start=False, stop=True,
                            )

                    # one contiguous tile [s15 | s25 | s35] so a single
                    # double-width DVE sub computes d=s15-s25 and e=s25-s35
                    # via overlapping slices
                    sall = spool.tile([STRIDE_, 3 * W_], f16, tag="sall")
                    nc.scalar.activation(sall[:M, 0:W_], ps[:M], Sig, bias=b15[:M], scale=10.0)
                    nc.scalar.activation(sall[:M, W_ : 2 * W_], ps[:M], Sig, bias=b25[:M], scale=10.0)
                    nc.scalar.activation(sall[:M, 2 * W_ : 3 * W_], ps[:M], Sig, bias=b35[:M], scale=10.0)

                    # Software pipelining: the sub/mul/add/out for the
                    # PREVIOUS strip are emitted here so no DVE-stream wait
                    # chains across pipeline stages.
                    if pending is not None:
                        emit_tail(pending)
                    pending = (sall, xf, M, b, r0)

            if pending is not None:
                emit_tail(pending)

    nc.compile()
    if (b_per, h, w, stride) == (B_PER, H, W, STRIDE):
        _cached_nc = nc
    return nc


def run(x, trace=False):
    """Run the SPMD kernel on 8 cores. Returns (out_fp32, BassKernelResults)."""
    from concourse.bass_utils import run_bass_kernel_spmd

    nc = _build()
    x = np.asarray(x, dtype=np.float16)  # host-side cast: halves input DMA
    assert x.shape == (B, H, W), x.shape
    in_maps = [{"x": x[B_PER * c : B_PER * (c + 1)]} for c in range(N_CORES)]
    res = run_bass_kernel_spmd(nc, in_maps, core_ids=list(range(N_CORES)), trace=trace)
    out = np.concatenate(
        [res.results[c]["y"].astype(np.float32) for c in range(N_CORES)], axis=0
    )
    return out, res


def kernel(x):
    out, _ = run(x, trace=False)
    return out


# revision 20
# speedup vs baseline: 1.6075x; 1.0113x over previous
"""Continuous Game-of-Life Trainium2 kernel (v13: FWL-padded bands).

Reference computation (per batch image, cyclic 3x3 stencil):
    around = 8-neighbor sum of x (torus wrap)
    survive = sigmoid(10(around-1.5)) * sigmoid(10(3.5-around))
    birth   = sigmoid(10(around-2.5)) * sigmoid(10(3.5-around))
    out     = x*survive + (1-x)*birth

Algebraic simplification (BETA=10 transitions are >= 1.0 apart):
    s_c := sigmoid(10*around - 10*c)
    out ~= x*(s1.5 - s2.5) + (s2.5 - s3.5)    (max abs err 4.5e-5)

Optimization history (each step trace-verified on HW):
  v7  456us: SWDGE fp32 input + per-strip 1-row halo DMAs; stalled in
      17-51us chunks with the input stream latency-bound.
  v8c 343us: one contiguous 127-row SWDGE DMA per strip; top halo filled
      by an 8KB SBUF->SBUF copy from the previous strip's tile on the
      scalar HWDGE ring (its wait is pre-satisfied when ACT reaches it,
      and descriptor-gen overlaps sigmoid execution).  Output on the
      sync ring.  NOTE: big DRAM->SBUF transfers MUST be SWDGE
      (nc.gpsimd) -- the HWDGE path lands the whole transfer on a
      single SDMA engine (~27 GB/s; measured 1.35ms kernel).
  v9  281us: host pre-casts x to fp16 -- halves input DMA bytes and
      deletes the on-chip DVE cast (on-chip math is bit-identical).
  v11 277us: software-pipelined tail (sub/mul/add/out of strip t-1
      emitted after strip t's sigmoids) + deeper pools.  A PWL-on-DVE
      s15 variant was tried and REVERTED: any DVE op reading PSUM
      closes a PE<->DVE cycle through PSUM bank recycling (540us).
      Offloading the final add to GPSIMD also regressed (Q7 tensor ops
      run at ~0.42 efficiency; 304us).
  v13 274us (final): stationary bands zero-padded from 126 to 128
      columns (FWL wants NumWeights==128) and the pre-touch matmul's
      stationary switched to fp16 (fp32 matmuls disable FWL for the
      next load).  Also tried and REVERTED: N=1024/2048 matmuls (walrus
      s3d3_mm_num_elements caps matmul free dim at 512 = one PSUM
      bank), fpool=6 (re-starved the input stream: 436us), and HWDGE
      input (single-engine: 1.35ms).

Remaining structure (trace): ACT runs 3x1.9us sigmoids per strip and
idles ~1.7us waiting for the strip's last matmul (first-sigmoid start
== last-matmul end +- 50ns, every gapped strip); half the strips run
matmuls at ~2x cycle cost (PSUM-bankset-correlated, cause unknown).
Input-prefetch ramp is ~29us (SDMA round-robin delivers the first N
prefetched strips together).  An s15 hard-sigmoid on DVE (rel err
3.6e-3, validated) balances engine budgets on paper but any DVE op
reading PSUM recycles banks against PE and regressed 2x in practice.

Per-strip engines (steady state ~6us/strip):
  - TensorE: 8-neighbor sum via banded fp16 matmuls accumulated in
    PSUM, grouped by stationary operand (m0 vertical-only on the center
    columns, then m1 3-tap on the left/right shifted column views).
  - ScalarE: three sigmoids straight out of PSUM (scale/bias fused),
    ~1.9us each -- the throughput wall of this kernel.
  - VectorE: double-width fp16 sub, mul, add.
  - DMA out: fp16 (host upcasts to fp32).

Sharding: pure data-parallel over batch: 16 images -> 8 cores x 2 images.
The torus wrap is per-image so there is no cross-core halo at all.
"""

import numpy as np

B, H, W = 16, 2048, 2048
N_CORES = 8
B_PER = B // N_CORES  # 2 images per core
STRIDE = 126  # output rows per strip (128 input rows incl. halos)
N_STRIPS = (H + STRIDE - 1) // STRIDE  # 17
NBANKS = W // 512  # PSUM 512-col chunks per strip
PCOLS = 128  # stationary free dim, zero-padded to 128 so FWL engages

_cached_nc = None


def _band_matrices(m, dtype=np.float16):
    """[m+2, 128] stationary operands for the vertical taps.

    Tile layout: partitions 0..m-1 hold image rows r0..r0+m-1 (the cells),
    partition m holds the bottom halo row r0+m, partition m+1 holds the top
    halo row r0-1.  For output row p the vertical neighbors are partitions
    p-1 (or m+1 when p==0) and p+1.

    m0[k, p] = 1 for the two vertical neighbors (no center),
    m1[k, p] = 1 for the full 3-tap (used on the column-shifted views).
    Columns m..127 are zero padding (garbage PSUM rows m..127): FWL
    (2-elements-per-read weight load) only engages at 128 columns.
    """
    m0 = np.zeros((m + 2, PCOLS), dtype)
    m1 = np.zeros((m + 2, PCOLS), dtype)
    for p in range(m):
        up = m + 1 if p == 0 else p - 1
        m0[up, p] = 1.0
        m0[p + 1, p] = 1.0
        m1[up, p] = 1.0
        m1[p, p] = 1.0
        m1[p + 1, p] = 1.0
    return m0, m1


def _build(b_per=B_PER, h=H, w=W, stride=STRIDE):
    global _cached_nc
    if _cached_nc is not None and (b_per, h, w, stride) == (B_PER, H, W, STRIDE):
        return _cached_nc

    import concourse.mybir as mybir
    from concourse.bacc import Bacc
    from concourse.tile import TileContext

    B_PER_, H_, W_, STRIDE_ = b_per, h, w, stride
    N_STRIPS_ = (H_ + STRIDE_ - 1) // STRIDE_
    NBANKS_ = W_ // 512
    KROWS = STRIDE_ + 2  # input rows per full strip

    f32 = mybir.dt.float32
    f16 = mybir.dt.float16
    Sig = mybir.ActivationFunctionType.Sigmoid

    nc = Bacc(trn_type="TRN2")
    x_d = nc.dram_tensor("x", [B_PER_, H_, W_], f16, kind="ExternalInput")
    y_d = nc.dram_tensor("y", [B_PER_, H_, W_], f16, kind="ExternalOutput")

    consts = {}
    for m in sorted({STRIDE_, H_ - STRIDE_ * (N_STRIPS_ - 1)}):
        m0_np, m1_np = _band_matrices(m)
        consts[m] = (
            nc.inline_tensor(m0_np, f"m0_const_{m}"),
            nc.inline_tensor(m1_np, f"m1_const_{m}"),
        )

    with TileContext(nc) as tc:
        with (
            tc.tile_pool(name="wpool", bufs=1) as wpool,
            tc.tile_pool(name="fpool", bufs=10) as fpool,
            tc.tile_pool(name="spool", bufs=4) as spool,
            tc.tile_pool(name="dpool", bufs=4) as dpool,
            tc.tile_pool(name="mpool", bufs=3) as mpool,
            tc.tile_pool(name="opool", bufs=6) as opool,
            tc.tile_pool(name="ppool", bufs=2, space="PSUM") as ppool,
        ):
            bands = {}
            for m, (m0_d, m1_d) in consts.items():
                m0 = wpool.tile([m + 2, PCOLS], f16, name=f"m0_{m}")
                m1 = wpool.tile([m + 2, PCOLS], f16, name=f"m1_{m}")
                nc.sync.dma_start(out=m0[:], in_=m0_d[:])
                nc.sync.dma_start(out=m1[:], in_=m1_d[:])
                bands[m] = (m0, m1)

            # activation biases must be [128,1] APs, not immediates
            b15 = wpool.tile([128, 1], f32)
            b25 = wpool.tile([128, 1], f32)
            b35 = wpool.tile([128, 1], f32)
            nc.vector.memset(b15[:], -15.0)
            nc.vector.memset(b25[:], -25.0)
            nc.vector.memset(b35[:], -35.0)

            prev_xf = None
            pending = None  # (sall, xf, M, b, r0) awaiting sub/mul/add/out

            def emit_tail(p):
                sall, xf, M, b, r0 = p
                de = dpool.tile([STRIDE_, 2 * W_], f16, tag="de")
                nc.vector.tensor_sub(
                    out=de[:M], in0=sall[:M, 0 : 2 * W_], in1=sall[:M, W_ : 3 * W_]
                )
                m_t = mpool.tile([STRIDE_, W_], f16, tag="m")
                o = opool.tile([STRIDE_, W_], f16, tag="o")
                nc.vector.tensor_mul(out=m_t[:M], in0=xf[:M, :], in1=de[:M, 0:W_])
                nc.vector.tensor_add(out=o[:M], in0=m_t[:M], in1=de[:M, W_ : 2 * W_])
                nc.sync.dma_start(out=y_d[b, r0 : r0 + M, :], in_=o[:M])

            for b in range(B_PER_):
                for t in range(N_STRIPS_):
                    r0 = t * STRIDE_  # first output row
                    M = min(STRIDE_, H_ - r0)  # output rows this strip
                    k = M + 2  # partitions used (cells + 2 halos)
                    m0, m1 = bands[M]

                    # fp16 tile (host pre-casts x), partitions 0..M-1 =
                    # cells (rows r0..), partition M = bottom halo, M+1 =
                    # top halo.
                    # Allocated double-width so every buffer is 8KB-
                    # aligned per partition: at 4KB the odd-indexed pool
                    # buffers straddle SBUF banks and the PE's moving-
                    # operand reads run ~2x slow on alternating strips.
                    xft = fpool.tile([KROWS, 2 * W_], f16, tag="xf")
                    xf = xft[:, 0:W_]
                    if r0 + M < H_:
                        # cells + bottom halo: one contiguous SWDGE DMA
                        # (HWDGE DRAM->SBUF lands on a single SDMA engine
                        # -- measured 27 GB/s; SWDGE sprays all 16).
                        nc.gpsimd.dma_start(
                            out=xf[0 : M + 1, :], in_=x_d[b, r0 : r0 + M + 1, :]
                        )
                    else:
                        # last strip: bottom halo wraps to row 0
                        nc.gpsimd.dma_start(out=xf[0:M, :], in_=x_d[b, r0:H_, :])
                        nc.gpsimd.dma_start(out=xf[M : M + 1, :], in_=x_d[b, 0:1, :])
                    if t == 0:
                        # top halo wraps to the last image row (rare: 2/image)
                        nc.gpsimd.dma_start(
                            out=xf[M + 1 : M + 2, :], in_=x_d[b, H_ - 1 : H_, :]
                        )
                    else:
                        # top halo row r0-1 = previous strip's partition
                        # STRIDE-1: an 8KB SBUF->SBUF copy on the scalar
                        # HWDGE ring.  Its wait (previous strip's input
                        # landed) is long satisfied when the ACT queue
                        # reaches it, and its descriptor-gen overlaps
                        # sigmoid execution, so it costs nothing.
                        nc.scalar.dma_start(
                            out=xf[M + 1 : M + 2, :],
                            in_=prev_xf[STRIDE_ - 1 : STRIDE_, :],
                        )
                    prev_xf = xf

                    ps = ppool.tile([PCOLS, W_], f32, tag="ps")
                    m0s = m0[:k, :PCOLS]
                    m1s = m1[:k, :PCOLS]

                    # Pre-touch: a 1x1 matmul absorbs the PSUM-release wait
                    # (Matmult carries at most ONE sync wait; without this,
                    # Bacc's wait-merging couples strip t to strip t-1's
                    # activations and serializes PE behind ACT).
                    nc.tensor.matmul(
                        ps[:1, 0:1], m0[:1, :1], m0[:1, :1],
                        start=True, stop=True,
                    )

                    # around = 8-neighbor sum accumulated in PSUM, grouped
                    # by stationary operand to minimize weight switches.
                    # (N=512 is a hard matmul limit: wider PSUM APs fail
                    # walrus codegen's s3d3_mm_num_elements check.)
                    # m0 group: center column, vertical neighbors only.
                    for nb in range(NBANKS_):
                        c0 = nb * 512
                        nc.tensor.matmul(
                            ps[:PCOLS, c0 : c0 + 512], m0s, xf[:k, c0 : c0 + 512],
                            start=True, stop=False,
                        )
                    # m1 group, left-neighbor column: out col j += band @ x col j-1
                    for nb in range(NBANKS_):
                        c0 = nb * 512
                        c1 = c0 + 512
                        if nb == 0:
                            nc.tensor.matmul(
                                ps[:PCOLS, 1:512], m1s, xf[:k, 0:511],
                                start=False, stop=False,
                            )
                            nc.tensor.matmul(
                                ps[:PCOLS, 0:1], m1s, xf[:k, W_ - 1 : W_],
                                start=False, stop=False,
                            )
                        else:
                            nc.tensor.matmul(
                                ps[:PCOLS, c0:c1], m1s, xf[:k, c0 - 1 : c1 - 1],
                                start=False, stop=False,
                            )
                    # m1 group, right-neighbor column: out col j += band @ x col j+1
                    for nb in range(NBANKS_):
                        c0 = nb * 512
                        c1 = c0 + 512
                        if nb == NBANKS_ - 1:
                            nc.tensor.matmul(
                                ps[:PCOLS, c0 : W_ - 1], m1s, xf[:k, c0 + 1 : W_],
                                start=False, stop=False,
                            )
                            nc.tensor.matmul(
                                ps[:PCOLS, W_ - 1 : W_], m1s, xf[:k, 0:1],
                                start=False, stop=True,
                            )
                        else:
                            nc.tensor.matmul(
                                ps[:PCOLS, c0:c1], m1s, xf[:k, c0 + 1 : c1 + 1],
                                start=False, stop=True,
                            )

                    # one contiguous tile [s15 | s25 | s35] so a single
                    # double-width DVE sub computes d=s15-s25 and e=s25-s35
                    # via overlapping slices
                    sall = spool.tile([STRIDE_, 3 * W_], f16, tag="sall")
                    nc.scalar.activation(sall[:M, 0:W_], ps[:M], Sig, bias=b15[:M], scale=10.0)
                    nc.scalar.activation(sall[:M, W_ : 2 * W_], ps[:M], Sig, bias=b25[:M], scale=10.0)
                    nc.scalar.activation(sall[:M, 2 * W_ : 3 * W_], ps[:M], Sig, bias=b35[:M], scale=10.0)

                    # Software pipelining: the sub/mul/add/out for the
                    # PREVIOUS strip are emitted here so no DVE-stream wait
                    # chains across pipeline stages.
                    if pending is not None:
                        emit_tail(pending)
                    pending = (sall, xf, M, b, r0)

            if pending is not None:
                emit_tail(pending)

    nc.compile()
    if (b_per, h, w, stride) == (B_PER, H, W, STRIDE):
        _cached_nc = nc
    return nc


def run(x, trace=False):
    """Run the SPMD kernel on 8 cores. Returns (out_fp32, BassKernelResults)."""
    from concourse.bass_utils import run_bass_kernel_spmd

    nc = _build()
    x = np.asarray(x, dtype=np.float16)  # host-side cast: halves input DMA
    assert x.shape == (B, H, W), x.shape
    in_maps = [{"x": x[B_PER * c : B_PER * (c + 1)]} for c in range(N_CORES)]
    res = run_bass_kernel_spmd(nc, in_maps, core_ids=list(range(N_CORES)), trace=trace)
    out = np.concatenate(
        [res.results[c]["y"].astype(np.float32) for c in range(N_CORES)], axis=0
    )
    return out, res


def kernel(x):
    out, _ = run(x, trace=False)
    return out
